# revision 15
# baseline (speedup 1.0000x reference)
# Trainium2 Bass kernel for nn_AttnModel_64098091926054.
#
# Strategy: pure data parallel over batch (256 boards -> 32 per core x 8 cores).
# Host-side constant folding (softmax shift-invariance kills the x-dependent
# k-term; q_w folds into qk_w (512x19); kvx_v/Wv fold through fin_w).
#
# v4: latency-oriented. The kernel is one serial dependency loop per layer:
#   t1 -> dots -> exp -> t2 -> s4 -> s4n -> sT -> sfin -> tv -> x_mid -> fc0
#   -> relu -> (u@fq) -> g -> t1' ...
# so every link is minimized:
#  - Transposed compute: residual x^T (128, 4x32) pure bf16; weights are the
#    stationary operand (fp8e4), skinny bf16 activations stream (N=32).
#  - Pipelined attention: g_{l+1} = x_mid@qk + u~@(sign(a)*fc1@qk) + const
#    (fc1@qk folded on host) => fc1 matmuls + residual hide under attention.
#  - Group-major 4-way cell split: 81 cells -> 4 groups x 21 (3 pads), so
#    all 128 partitions work and DVE free sizes shrink 27%. Pad cells carry
#    a -40 in the padded j-column against a constant 1.0 in g3's pad column,
#    so exp gives them zero weight with zero extra instructions.
#  - g3 (128, 19) comes from ONE matmul set with a step-0-broadcast AP on
#    the stationary operand (x columns replicated 4x) - no replicate matmul,
#    no g_sb copy.
#  - fc0/fc1 biases enter PSUM via early K=1 rank-1 matmuls; |alpha| rides
#    in a SINGLE (128,128) relu (scale imm); fin bias rides in the K=20
#    sfin matmul; fc1 residual is a single TT add/sub.
#  - softmax normalization (rs/recip/r3) runs on accum_out + PE and hides
#    under t2/s4; s4 is normalized instead of e.

import numpy as np
import ml_dtypes

import concourse.bass as bass
import concourse.bacc as bacc
import concourse.mybir as mybir
import concourse.tile as tile
from concourse.bass_utils import run_bass_kernel_spmd

BS, D, L, B, P, POSD, J = 9, 512, 8, 256, 81, 12, 19
NCORES = 8
NB = B // NCORES          # 32 boards per core
GG, PQ = 4, 21            # 84 = 4 groups x 21 cells (3 pads)
NP = 128                  # partitions: p = gg*32 + b  (group-major)
JP = J + 1                # 20: j padded (pad col doubles as -40 pad-cell lane)
PQP = PQ + 1              # 22: pq padded even
NEGBIG = -40.0
OFFSETS = [(-1, 0), (-1, 1), (0, -1), (0, 0), (0, 1), (-1, -1), (-1, 0)]

f32 = mybir.dt.float32
bf16 = mybir.dt.bfloat16
fp8 = mybir.dt.float8e4
bf16_np = ml_dtypes.bfloat16
fp8_np = ml_dtypes.float8_e4m3

# cpk16 columns (bf16): bfpj (21x20) | bfjp (19x22) | wpost | e2p | e2tp
C_BFPJ = 0
C_BFJP = C_BFPJ + PQ * JP          # 420
C_POST = C_BFJP + J * PQP          # 838
C_E2 = C_POST + P                  # 919
C_E2T = C_E2 + NB                  # 951
C16_END = C_E2T + 128              # 1079
AluOp = mybir.AluOpType
Act = mybir.ActivationFunctionType


def _positions():
    lin = np.linspace(0.0, 1.0, BS, dtype=np.float32)
    rs, cs = np.meshgrid(lin, lin, indexing="ij")
    zs = (rs + cs) / 2.0
    xs = np.stack([rs, cs, zs], -1).astype(np.float32)
    feats = []
    for p in [4.0 / (BS - 1), 16.0 / (BS - 1)]:
        a = (2.0 * np.pi * xs / p).astype(np.float32)
        feats.append(np.concatenate([np.cos(a), np.sin(a)], -1).astype(np.float32))
    return np.concatenate(feats, -1)  # (9, 9, 12)


def _prepare(obs, pos):
    single = obs[..., 0] - obs[..., 1]
    aug = np.pad(single, ((0, 0), (1, 1), (1, 1)))
    w = aug.shape[-1]
    outs = [aug[:, 1 + r : w - 1 + r, 1 + c : w - 1 + c] for (r, c) in OFFSETS]
    neigh = np.stack(outs, -1)
    n = obs.shape[0]
    stack = np.concatenate(
        [neigh, np.broadcast_to(pos, (n,) + pos.shape)], -1
    ).astype(np.float32)
    return stack.reshape(n, P, J)  # (B, 81, 19)


def _fold(inp):
    """Host-side constant folding of weights. All f32 numpy, unscaled."""
    scale = np.float32(1.0 / np.sqrt(D))
    Wk = inp["kvb_w"][:, :, :D]                                   # (L,19,512)
    Wv = inp["kvb_w"][:, :, D:]
    kvx_v = inp["kvx_w"][:, :, D:]                                # (L,512,512)
    qk_w = np.einsum("ldh,ljh->ldj", inp["q_w"], Wk) * scale      # (L,512,19)
    qk_b = np.einsum("lh,ljh->lj", inp["q_b"], Wk) * scale        # (L,19)
    afin = np.einsum("lde,leh->ldh", kvx_v, inp["fin_w"])         # (L,512,512)
    sfin = np.einsum("lje,leh->ljh", Wv, inp["fin_w"])            # (L,19,512)
    bias_v = inp["kvx_b"][:, D:] + inp["kvb_b"][:, D:]
    cfin = np.einsum("le,leh->lh", bias_v, inp["fin_w"]) + inp["fin_b"]
    return qk_w, qk_b, afin, sfin, cfin


def _ktile_lhsT(W):
    """(L,512,512) -> (L,128,2048) with col ((o*4+kt)*128+m) = W[l,kt*128+k,o*128+m]."""
    Lx = W.shape[0]
    return np.ascontiguousarray(
        W.reshape(Lx, 4, 128, 4, 128).transpose(0, 2, 3, 1, 4).reshape(Lx, 128, 2048)
    )


def _build_nc(alpha):
    nc = bacc.Bacc("TRN2", target_bir_lowering=False, debug=False)

    d_cpk16 = nc.dram_tensor("cpk16", [128, C16_END], bf16, kind="ExternalInput")
    d_e2pf = nc.dram_tensor("e2pf", [128, NB], f32, kind="ExternalInput")
    d_wbig = nc.dram_tensor("wbig", [L, 128, 3 * 2048], fp8, kind="ExternalInput")
    # per layer: qk k-tiles (4*19) then fq k-tiles (4*19)
    d_qkfq = nc.dram_tensor("qkfq", [128, L * 8 * J], bf16, kind="ExternalInput")
    d_sfall = nc.dram_tensor("sfall", [JP, L * D], bf16, kind="ExternalInput")
    d_bias = nc.dram_tensor("bias", [1, L * 2 * D], bf16, kind="ExternalInput")
    d_gconst = nc.dram_tensor("gconst", [1, L * J], f32, kind="ExternalInput")
    d_whead = nc.dram_tensor("whead", [128, 4 * POSD], bf16, kind="ExternalInput")
    d_out = nc.dram_tensor("out", [NB, P], f32, kind="ExternalOutput")

    def bcast_mid(ap2d, n):
        # (p, k) AP -> (p, n, k) with step-0 broadcast in the middle
        return bass.AP(
            tensor=ap2d.tensor, offset=ap2d.offset,
            ap=[ap2d.ap[0], [0, n], ap2d.ap[1]],
        )

    with tile.TileContext(nc) as tc:
        with (
            tc.tile_pool(name="consts", bufs=1) as consts,
            tc.tile_pool(name="wpool", bufs=3) as wpool,
            tc.tile_pool(name="ap", bufs=3) as apool,
            tc.tile_pool(name="attn", bufs=1) as atp,
            tc.tile_pool(name="pm", bufs=4, space="PSUM") as pm,
            tc.tile_pool(name="pt", bufs=2, space="PSUM") as pt,
        ):
            # ---- constants (7 DMAs) ----
            cpk16 = consts.tile([128, C16_END], bf16)
            nc.sync.dma_start(out=cpk16, in_=d_cpk16[:, :])
            e2pf = consts.tile([128, NB], f32)
            nc.sync.dma_start(out=e2pf, in_=d_e2pf[:, :])
            qkfq = consts.tile([128, L * 8 * J], bf16)
            nc.sync.dma_start(out=qkfq, in_=d_qkfq[:, :])
            sfall = consts.tile([JP, L * D], bf16)
            nc.sync.dma_start(out=sfall, in_=d_sfall[:, :])
            biasall = consts.tile([1, L * 2 * D], bf16)
            nc.sync.dma_start(out=biasall, in_=d_bias[:, :])
            gconst = consts.tile([1, L * J], f32)
            nc.sync.dma_start(out=gconst, in_=d_gconst[:, :])
            whead = consts.tile([128, 4 * POSD], bf16)
            nc.sync.dma_start(out=whead, in_=d_whead[:, :])

            bfpj4 = cpk16[:NP, C_BFPJ:C_BFJP].rearrange("p (a b) -> p a b", b=JP)
            bfjp4 = cpk16[:NP, C_BFJP:C_POST].rearrange("p (a b) -> p a b", b=PQP)
            wpost = cpk16[:POSD, C_POST:C_E2]     # (12, 81) bf16
            e2p = cpk16[:128, C_E2:C_E2T]         # (128, 32) bf16 group-major
            e2tp = cpk16[:NB, C_E2T:C16_END]      # (32, 128) bf16

            ones1 = consts.tile([1, NB], f32)
            nc.vector.memset(ones1, 1.0)
            ones_bf = consts.tile([1, NB], bf16)
            nc.vector.memset(ones_bf, 1.0)
            # persistent attention buffers (serial chain -> single-buffered)
            sT_buf = consts.tile([JP, NB], bf16)
            nc.vector.memset(sT_buf, 1.0)   # row 19 stays 1.0 (cfin ones row)
            g3 = consts.tile([NP, JP], bf16)
            nc.vector.memset(g3, 1.0)       # pad col 19 stays 1.0 (-40 lane)
            e4 = consts.tile([NP, PQP], bf16)
            nc.vector.memset(e4, 0.0)       # pad col 21 stays 0
            g_sb = atp.tile([NB, J], bf16, tag="gsb")
            s4 = atp.tile([NP, J], bf16, tag="s4")
            rs = atp.tile([NP, 1], f32, tag="rs")
            t1 = atp.tile([NP, PQ * JP], bf16, tag="t1")
            t2 = atp.tile([NP, J * PQP], bf16, tag="t2")
            dots = atp.tile([NP, PQ], f32, tag="dots")
            s4n = atp.tile([NP, J], bf16, tag="s4n")
            recip = atp.tile([NB, 1], bf16, tag="recip")

            # residual stream: x^T as (128, 4*32), pure bf16
            xT_b = apool.tile([128, 4 * NB], bf16, tag="xb")
            nc.vector.memset(xT_b, 0.0)
            xT_mid = xT_b

            def xsl(t, kt):
                return t[:, kt * NB : (kt + 1) * NB]

            t1_3 = t1[:, :].rearrange("p (a b) -> p a b", b=JP)
            t2_3 = t2[:, :].rearrange("p (a b) -> p a b", b=PQP)

            prev = None  # (wb, uT, y_ps, sign) of layer l-1 pending fc1
            for l in range(L):
                wb = wpool.tile([128, 3 * 2048], fp8, tag="wb")
                nc.sync.dma_start(out=wb, in_=d_wbig[l, :, :])

                def wtile(mat, o, kt, wbx=None):
                    wbx = wb if wbx is None else wbx
                    c = ((mat * 4 + o) * 4 + kt) * 128
                    return wbx[:, c : c + 128]

                qk_l = qkfq[:, l * 8 * J : l * 8 * J + 4 * J]
                fq_l = qkfq[:, l * 8 * J + 4 * J : (l + 1) * 8 * J]
                aab = abs(alpha[l])

                # ---- g (32,19) then replicate to 128 group-major rows ----
                g_ps = pt.tile([NB, J], f32, tag="sp")
                nc.tensor.matmul(
                    g_ps, ones1, gconst[:, l * J : (l + 1) * J],
                    start=True, stop=(l == 0),
                )
                if l > 0:
                    wb_p, uT_p, y_p, sgn_p = prev
                    for kt in range(4):
                        nc.tensor.matmul(
                            g_ps, xsl(xT_mid, kt),
                            qk_l[:, kt * J : (kt + 1) * J],
                            start=False, stop=False,
                        )
                    for kt in range(4):
                        nc.tensor.matmul(
                            g_ps, xsl(uT_p, kt),
                            fq_l[:, kt * J : (kt + 1) * J],
                            start=False, stop=(kt == 3),
                        )
                nc.vector.tensor_copy(g_sb, g_ps)
                g3_ps = pt.tile([NP, J], f32, tag="sp")
                nc.tensor.matmul(g3_ps, e2tp, g_sb, start=True, stop=True)
                nc.scalar.activation(g3[:, 0:J], g3_ps, Act.Copy)

                # ---- deferred fc1 of layer l-1 (hides under attention) ----
                if l > 0:
                    for o in range(4):
                        for kt in range(4):
                            nc.tensor.matmul(
                                xsl(y_p, o), wtile(2, o, kt, wb_p),
                                xsl(uT_p, kt),
                                start=False, stop=(kt == 3),
                            )

                # psum tiles + early bias rank-1s (no deps; run in PE slack)
                ft = pm.tile([128, 4 * NB], f32, tag="mm")
                u_ps = pm.tile([128, 4 * NB], f32, tag="mm")
                y_ps = pm.tile([128, 4 * NB], f32, tag="mm")
                for o in range(4):
                    nc.tensor.matmul(
                        xsl(u_ps, o),
                        biasall[:, l * 2 * D + o * 128 : l * 2 * D + (o + 1) * 128],
                        ones_bf, start=True, stop=False,
                    )
                    nc.tensor.matmul(
                        xsl(y_ps, o),
                        biasall[:, l * 2 * D + D + o * 128 : l * 2 * D + D + (o + 1) * 128],
                        ones_bf, start=True, stop=False,
                    )

                # ---- dots = bfeat . g -> (128, 21) ----
                nc.vector.tensor_tensor(
                    t1_3, bfpj4, bcast_mid(g3[:, :], PQ), op=AluOp.mult
                )
                nc.vector.tensor_reduce(
                    dots, t1_3, axis=mybir.AxisListType.X, op=AluOp.add
                )
                # fc1 residual of layer l-1 (DVE slot after dots, before t2)
                if l > 0:
                    nxb = apool.tile([128, 4 * NB], bf16, tag="xb")
                    nc.vector.tensor_tensor(
                        nxb, xT_mid, y_p,
                        op=AluOp.add if sgn_p >= 0 else AluOp.subtract,
                    )
                    xT_b = nxb
                    # fin + fc0 x-parts now that x_l is final
                    for o in range(4):
                        for kt in range(4):
                            nc.tensor.matmul(
                                xsl(ft, o), wtile(0, o, kt), xsl(xT_b, kt),
                                start=(kt == 0), stop=False,
                            )
                    for o in range(4):
                        for kt in range(4):
                            nc.tensor.matmul(
                                xsl(u_ps, o), wtile(1, o, kt), xsl(xT_b, kt),
                                start=False, stop=False,
                            )
                with nc.allow_low_precision(reason="softmax rowsum"):
                    nc.scalar.activation(
                        e4[:, 0:PQ], dots, Act.Exp, accum_out=rs
                    )
                rsb_ps = pt.tile([NB, 1], f32, tag="sp")
                nc.tensor.matmul(rsb_ps, e2pf, rs, start=True, stop=True)

                # ---- s4 = sum_p e[b,p] bfeat[b,p,j] (unnormalized) ----
                nc.vector.tensor_tensor(
                    t2_3, bfjp4, bcast_mid(e4[:, :], J), op=AluOp.mult
                )
                with nc.allow_low_precision(reason="softmax recip in bf16"):
                    nc.vector.reciprocal(recip, rsb_ps)
                r3_ps = pt.tile([128, 1], f32, tag="sp")
                nc.tensor.matmul(r3_ps, e2tp, recip, start=True, stop=True)
                with nc.allow_low_precision(reason="attention s in bf16"):
                    nc.vector.tensor_reduce(
                        s4, t2_3, axis=mybir.AxisListType.X, op=AluOp.add
                    )
                nc.vector.tensor_scalar_mul(s4n, s4, r3_ps[:, :])
                # group-sum to s^T directly: (19,32) = s4n.T @ e2
                sT_ps = pt.tile([J, NB], f32, tag="sp")
                nc.tensor.matmul(sT_ps, s4n, e2p, start=True, stop=True)
                nc.vector.tensor_copy(sT_buf[0:J, :], sT_ps)

                # ---- fin tail: += s @ [sfin;cfin] (K=20, bias inside) ----
                for o in range(4):
                    nc.tensor.matmul(
                        xsl(ft, o),
                        sfall[:, l * D + o * 128 : l * D + (o + 1) * 128],
                        sT_buf,
                        start=(l == 0), stop=True,
                    )
                # tv = alpha*relu(ft); fc0 tv-part rides on it so the
                # x_mid residual add leaves the critical chain
                tv = apool.tile([128, 4 * NB], bf16, tag="tv")
                nc.vector.tensor_scalar(
                    tv, ft, 0.0, float(alpha[l]), op0=AluOp.max, op1=AluOp.mult
                )
                for o in range(4):
                    for kt in range(4):
                        nc.tensor.matmul(
                            xsl(u_ps, o), wtile(1, o, kt), xsl(tv, kt),
                            start=False, stop=(kt == 3),
                        )
                nmid = apool.tile([128, 4 * NB], bf16, tag="xb")
                nc.vector.tensor_tensor(nmid, xT_b, tv, op=AluOp.add)
                xT_mid = nmid
                nuT = apool.tile([128, 4 * NB], bf16, tag="uT")
                nc.scalar.activation(nuT, u_ps, Act.Relu, scale=aab)
                uT = nuT
                # fc1 matmuls deferred to next layer (after its g matmuls)
                prev = (wb, uT, y_ps, 1.0 if alpha[l] >= 0 else -1.0)

            # flush last layer's fc1 + residual
            wb_p, uT_p, y_p, sgn_p = prev
            for o in range(4):
                for kt in range(4):
                    nc.tensor.matmul(
                        xsl(y_p, o), wtile(2, o, kt, wb_p), xsl(uT_p, kt),
                        start=False, stop=(kt == 3),
                    )
            nxb = apool.tile([128, 4 * NB], bf16, tag="xb")
            nc.vector.tensor_tensor(
                nxb, xT_mid, y_p,
                op=AluOp.add if sgn_p >= 0 else AluOp.subtract,
            )
            xT_b = nxb

            # ---- head: logits = log_softmax((x @ head_w) @ posT) ----
            zT_ps = pt.tile([POSD, NB], f32, tag="sp")
            for kt in range(4):
                nc.tensor.matmul(
                    zT_ps, whead[:, kt * POSD : (kt + 1) * POSD], xsl(xT_b, kt),
                    start=(kt == 0), stop=(kt == 3),
                )
            zT = apool.tile([POSD, NB], bf16, tag="zT")
            nc.vector.tensor_copy(zT, zT_ps)
            lg_ps = pt.tile([NB, P], f32, tag="sp")
            nc.tensor.matmul(lg_ps, zT, wpost, start=True, stop=True)
            lg = apool.tile([NB, P], f32, tag="lg")
            nc.scalar.activation(lg, lg_ps, Act.Copy)
            mx = apool.tile([NB, 1], f32, tag="mx")
            nc.vector.tensor_reduce(
                mx, lg[:, :], axis=mybir.AxisListType.X, op=AluOp.max
            )
            negmx = apool.tile([NB, 1], f32, tag="nmx")
            nc.vector.tensor_scalar_mul(negmx, mx, -1.0)
            ex = apool.tile([NB, P], f32, tag="ex")
            sume = apool.tile([NB, 1], f32, tag="sume")
            nc.scalar.activation(
                ex, lg, Act.Exp, bias=negmx[:, :], accum_out=sume
            )
            lse = apool.tile([NB, 1], f32, tag="lse")
            nc.scalar.activation(lse, sume, Act.Ln)
            c = apool.tile([NB, 1], f32, tag="c")
            nc.vector.tensor_add(c, mx, lse)
            outf = apool.tile([NB, P], f32, tag="outf")
            nc.vector.tensor_scalar(
                outf, lg[:, :], c[:, :], None, op0=AluOp.subtract
            )
            nc.sync.dma_start(out=d_out[:, :], in_=outf)

    nc.finalize()
    return nc


def kernel(**inputs):
    inp = {k: np.asarray(v, dtype=np.float32) for k, v in inputs.items()}
    pos = _positions()
    bfeat = _prepare(inp["obs"], pos)  # (256, 81, 19)
    qk_w, qk_b, afin, sfin, cfin = _fold(inp)
    alpha = inp["alpha"].astype(np.float32)

    wbig = np.concatenate(
        [_ktile_lhsT(afin), _ktile_lhsT(inp["fc0_w"]), _ktile_lhsT(inp["fc1_w"])],
        axis=2,
    ).astype(fp8_np)  # (L, 128, 6144)

    # g pipelining folds: fq_l = sign(a_{l-1}) * fc1_{l-1} @ qk_l
    fq = np.zeros((L, D, J), np.float32)
    gconst = qk_b.copy()
    for l in range(1, L):
        sgn = 1.0 if alpha[l - 1] >= 0 else -1.0
        fq[l] = sgn * (inp["fc1_w"][l - 1] @ qk_w[l])
        gconst[l] = qk_b[l] + alpha[l - 1] * (inp["fc1_b"][l - 1] @ qk_w[l])
    qkfq = np.zeros((128, L * 8 * J), np.float32)
    for l in range(L):
        qkfq[:, l * 8 * J : l * 8 * J + 4 * J] = (
            qk_w[l].reshape(4, 128, J).transpose(1, 0, 2).reshape(128, 4 * J)
        )
        qkfq[:, l * 8 * J + 4 * J : (l + 1) * 8 * J] = (
            fq[l].reshape(4, 128, J).transpose(1, 0, 2).reshape(128, 4 * J)
        )

    sfin_aug = np.concatenate([sfin, cfin[:, None, :]], axis=1)  # (L, 20, 512)
    sfall = np.ascontiguousarray(
        sfin_aug.transpose(1, 0, 2)
    ).reshape(JP, L * D).astype(bf16_np)

    biasall = np.zeros((1, L * 2 * D), np.float32)
    for l in range(L):
        biasall[0, l * 2 * D : l * 2 * D + D] = inp["fc0_b"][l]
        biasall[0, l * 2 * D + D : (l + 1) * 2 * D] = (
            (1.0 if alpha[l] >= 0 else -1.0) * alpha[l] * inp["fc1_b"][l]
        )
    gconst_v = gconst.reshape(1, L * J).astype(np.float32)
    whead = (
        inp["head_w"].reshape(4, 128, POSD).transpose(1, 0, 2)
        .reshape(128, 4 * POSD).astype(bf16_np)
    )

    # group-major constants: partition p = gg*32 + b, cell = gg*21 + pq
    e2 = np.zeros((NP, NB), np.float32)
    for gg in range(GG):
        for b in range(NB):
            e2[gg * NB + b, b] = 1.0

    in_maps = []
    for cc in range(NCORES):
        bf = bfeat[cc * NB : (cc + 1) * NB]          # (32, 81, 19)
        # pad cells 81..83 with zeros, cell c -> (gg=c//21, pq=c%21)
        bfp = np.zeros((NB, GG * PQ, J), np.float32)
        bfp[:, :P, :] = bf
        bfg = bfp.reshape(NB, GG, PQ, J).transpose(1, 0, 2, 3)  # (gg,b,pq,j)
        cpk = np.zeros((128, C16_END), np.float32)
        bfpj = np.zeros((GG, NB, PQ, JP), np.float32)
        bfpj[:, :, :, :J] = bfg
        # pad-cell kill switch: -40 in the j-pad lane (g3 pad col is 1.0)
        for c in range(P, GG * PQ):
            bfpj[c // PQ, :, c % PQ, J] = NEGBIG
        cpk[:, C_BFPJ:C_BFJP] = bfpj.reshape(NP, PQ * JP)
        bfjp = np.zeros((GG, NB, J, PQP), np.float32)
        bfjp[:, :, :, :PQ] = bfg.transpose(0, 1, 3, 2)
        cpk[:, C_BFJP:C_POST] = bfjp.reshape(NP, J * PQP)
        cpk[:POSD, C_POST:C_E2] = pos.reshape(P, POSD).T
        cpk[:, C_E2:C_E2T] = e2
        cpk[:NB, C_E2T:C16_END] = e2.T
        in_maps.append({
            "cpk16": cpk.astype(bf16_np), "e2pf": e2,
            "wbig": wbig, "qkfq": qkfq.astype(bf16_np), "sfall": sfall,
            "bias": biasall.astype(bf16_np), "gconst": gconst_v, "whead": whead,
        })

    nc = _build_nc([float(a) for a in alpha])
    res = run_bass_kernel_spmd(nc, in_maps, core_ids=list(range(NCORES)))
    out = np.concatenate([r["out"] for r in res.results], axis=0)  # (256, 81)
    return out.astype(np.float32)


# revision 19
# speedup vs baseline: 1.0064x; 1.0064x over previous
# Trainium2 Bass kernel for nn_AttnModel_64098091926054.
#
# Strategy: pure data parallel over batch (256 boards -> 32 per core x 8 cores).
# Host-side constant folding (softmax shift-invariance kills the x-dependent
# k-term; q_w folds into qk_w (512x19); kvx_v/Wv fold through fin_w).
#
# v4: latency-oriented. The kernel is one serial dependency loop per layer:
#   t1 -> dots -> exp -> t2 -> s4 -> s4n -> sT -> sfin -> tv -> x_mid -> fc0
#   -> relu -> (u@fq) -> g -> t1' ...
# so every link is minimized:
#  - Transposed compute: residual x^T (128, 4x32) pure bf16; weights are the
#    stationary operand (fp8e4), skinny bf16 activations stream (N=32).
#  - Pipelined attention: g_{l+1} = x_mid@qk + u~@(sign(a)*fc1@qk) + const
#    (fc1@qk folded on host) => fc1 matmuls + residual hide under attention.
#  - Group-major 4-way cell split: 81 cells -> 4 groups x 21 (3 pads), so
#    all 128 partitions work and DVE free sizes shrink 27%. Pad cells carry
#    a -40 in the padded j-column against a constant 1.0 in g3's pad column,
#    so exp gives them zero weight with zero extra instructions.
#  - g3 (128, 19) comes from ONE matmul set with a step-0-broadcast AP on
#    the stationary operand (x columns replicated 4x) - no replicate matmul,
#    no g_sb copy.
#  - fc0/fc1 biases enter PSUM via early K=1 rank-1 matmuls; |alpha| rides
#    in a SINGLE (128,128) relu (scale imm); fin bias rides in the K=20
#    sfin matmul; fc1 residual is a single TT add/sub.
#  - softmax normalization (rs/recip/r3) runs on accum_out + PE and hides
#    under t2/s4; s4 is normalized instead of e.

import numpy as np
import ml_dtypes

import concourse.bass as bass
import concourse.bacc as bacc
import concourse.mybir as mybir
import concourse.tile as tile
from concourse.bass_utils import run_bass_kernel_spmd

BS, D, L, B, P, POSD, J = 9, 512, 8, 256, 81, 12, 19
NCORES = 8
NB = B // NCORES          # 32 boards per core
GG, PQ = 4, 21            # 84 = 4 groups x 21 cells (3 pads)
NP = 128                  # partitions: p = gg*32 + b  (group-major)
JP = J + 1                # 20: j padded (pad col doubles as -40 pad-cell lane)
PQP = PQ + 1              # 22: pq padded even
NEGBIG = -40.0
OFFSETS = [(-1, 0), (-1, 1), (0, -1), (0, 0), (0, 1), (-1, -1), (-1, 0)]

f32 = mybir.dt.float32
bf16 = mybir.dt.bfloat16
fp8 = mybir.dt.float8e4
bf16_np = ml_dtypes.bfloat16
fp8_np = ml_dtypes.float8_e4m3

# cpk16 columns (bf16): bfpj (21x20) | bfjp (19x22) | wpost | e2p | e2tp
C_BFPJ = 0
C_BFJP = C_BFPJ + PQ * JP          # 420
C_POST = C_BFJP + J * PQP          # 838
C_E2 = C_POST + P                  # 919
C_E2T = C_E2 + NB                  # 951
C16_END = C_E2T + 128              # 1079
AluOp = mybir.AluOpType
Act = mybir.ActivationFunctionType


def _positions():
    lin = np.linspace(0.0, 1.0, BS, dtype=np.float32)
    rs, cs = np.meshgrid(lin, lin, indexing="ij")
    zs = (rs + cs) / 2.0
    xs = np.stack([rs, cs, zs], -1).astype(np.float32)
    feats = []
    for p in [4.0 / (BS - 1), 16.0 / (BS - 1)]:
        a = (2.0 * np.pi * xs / p).astype(np.float32)
        feats.append(np.concatenate([np.cos(a), np.sin(a)], -1).astype(np.float32))
    return np.concatenate(feats, -1)  # (9, 9, 12)


def _prepare(obs, pos):
    single = obs[..., 0] - obs[..., 1]
    aug = np.pad(single, ((0, 0), (1, 1), (1, 1)))
    w = aug.shape[-1]
    outs = [aug[:, 1 + r : w - 1 + r, 1 + c : w - 1 + c] for (r, c) in OFFSETS]
    neigh = np.stack(outs, -1)
    n = obs.shape[0]
    stack = np.concatenate(
        [neigh, np.broadcast_to(pos, (n,) + pos.shape)], -1
    ).astype(np.float32)
    return stack.reshape(n, P, J)  # (B, 81, 19)


def _fold(inp):
    """Host-side constant folding of weights. All f32 numpy, unscaled."""
    scale = np.float32(1.0 / np.sqrt(D))
    Wk = inp["kvb_w"][:, :, :D]                                   # (L,19,512)
    Wv = inp["kvb_w"][:, :, D:]
    kvx_v = inp["kvx_w"][:, :, D:]                                # (L,512,512)
    qk_w = np.einsum("ldh,ljh->ldj", inp["q_w"], Wk) * scale      # (L,512,19)
    qk_b = np.einsum("lh,ljh->lj", inp["q_b"], Wk) * scale        # (L,19)
    afin = np.einsum("lde,leh->ldh", kvx_v, inp["fin_w"])         # (L,512,512)
    sfin = np.einsum("lje,leh->ljh", Wv, inp["fin_w"])            # (L,19,512)
    bias_v = inp["kvx_b"][:, D:] + inp["kvb_b"][:, D:]
    cfin = np.einsum("le,leh->lh", bias_v, inp["fin_w"]) + inp["fin_b"]
    return qk_w, qk_b, afin, sfin, cfin


def _ktile_lhsT(W):
    """(L,512,512) -> (L,128,2048) with col ((o*4+kt)*128+m) = W[l,kt*128+k,o*128+m]."""
    Lx = W.shape[0]
    return np.ascontiguousarray(
        W.reshape(Lx, 4, 128, 4, 128).transpose(0, 2, 3, 1, 4).reshape(Lx, 128, 2048)
    )


def _build_nc(alpha):
    nc = bacc.Bacc("TRN2", target_bir_lowering=False, debug=False)

    d_cpk16 = nc.dram_tensor("cpk16", [128, C16_END], bf16, kind="ExternalInput")
    d_e2pf = nc.dram_tensor("e2pf", [128, NB], f32, kind="ExternalInput")
    d_wbig = nc.dram_tensor("wbig", [L, 128, 3 * 2048], fp8, kind="ExternalInput")
    # per layer: qk k-tiles (4*19) then fq k-tiles (4*19)
    d_qkfq = nc.dram_tensor("qkfq", [128, L * 8 * J], bf16, kind="ExternalInput")
    d_sfall = nc.dram_tensor("sfall", [JP, L * D], bf16, kind="ExternalInput")
    d_bias = nc.dram_tensor("bias", [1, L * 2 * D], bf16, kind="ExternalInput")
    d_gconst = nc.dram_tensor("gconst", [1, L * J], f32, kind="ExternalInput")
    d_whead = nc.dram_tensor("whead", [128, 4 * POSD], bf16, kind="ExternalInput")
    d_xmid0 = nc.dram_tensor("xmid0", [128, 4 * NB], bf16, kind="ExternalInput")
    d_out = nc.dram_tensor("out", [NB, P], f32, kind="ExternalOutput")

    def bcast_mid(ap2d, n):
        # (p, k) AP -> (p, n, k) with step-0 broadcast in the middle
        return bass.AP(
            tensor=ap2d.tensor, offset=ap2d.offset,
            ap=[ap2d.ap[0], [0, n], ap2d.ap[1]],
        )

    with tile.TileContext(nc) as tc:
        with (
            tc.tile_pool(name="consts", bufs=1) as consts,
            tc.tile_pool(name="wpool", bufs=8) as wpool,
            tc.tile_pool(name="ap", bufs=3) as apool,
            tc.tile_pool(name="attn", bufs=1) as atp,
            tc.tile_pool(name="pm", bufs=4, space="PSUM") as pm,
            tc.tile_pool(name="pt", bufs=2, space="PSUM") as pt,
        ):
            # ---- constants (7 DMAs) ----
            cpk16 = consts.tile([128, C16_END], bf16)
            nc.sync.dma_start(out=cpk16, in_=d_cpk16[:, :])
            e2pf = consts.tile([128, NB], f32)
            nc.sync.dma_start(out=e2pf, in_=d_e2pf[:, :])
            qkfq = consts.tile([128, L * 8 * J], bf16)
            nc.sync.dma_start(out=qkfq, in_=d_qkfq[:, :])
            sfall = consts.tile([JP, L * D], bf16)
            nc.sync.dma_start(out=sfall, in_=d_sfall[:, :])
            biasall = consts.tile([1, L * 2 * D], bf16)
            nc.sync.dma_start(out=biasall, in_=d_bias[:, :])
            gconst = consts.tile([1, L * J], f32)
            nc.sync.dma_start(out=gconst, in_=d_gconst[:, :])
            whead = consts.tile([128, 4 * POSD], bf16)
            nc.sync.dma_start(out=whead, in_=d_whead[:, :])
            xmid0 = consts.tile([128, 4 * NB], bf16)
            nc.sync.dma_start(out=xmid0, in_=d_xmid0[:, :])

            bfpj4 = cpk16[:NP, C_BFPJ:C_BFJP].rearrange("p (a b) -> p a b", b=JP)
            bfjp4 = cpk16[:NP, C_BFJP:C_POST].rearrange("p (a b) -> p a b", b=PQP)
            wpost = cpk16[:POSD, C_POST:C_E2]     # (12, 81) bf16
            e2p = cpk16[:128, C_E2:C_E2T]         # (128, 32) bf16 group-major
            e2tp = cpk16[:NB, C_E2T:C16_END]      # (32, 128) bf16

            ones1 = consts.tile([1, NB], f32)
            nc.vector.memset(ones1, 1.0)
            ones_bf = consts.tile([1, NB], bf16)
            nc.vector.memset(ones_bf, 1.0)
            # persistent attention buffers (serial chain -> single-buffered)
            sT_buf = consts.tile([JP, NB], bf16)
            nc.vector.memset(sT_buf, 1.0)   # row 19 stays 1.0 (cfin ones row)
            g3 = consts.tile([NP, JP], bf16)
            nc.vector.memset(g3, 1.0)       # pad col 19 stays 1.0 (-40 lane)
            e4 = consts.tile([NP, PQP], bf16)
            nc.vector.memset(e4, 0.0)       # pad col 21 stays 0
            g_sb = atp.tile([NB, J], bf16, tag="gsb")
            s4 = atp.tile([NP, J], bf16, tag="s4")
            rs = atp.tile([NP, 1], f32, tag="rs")
            t1 = atp.tile([NP, PQ * JP], bf16, tag="t1")
            t2 = atp.tile([NP, J * PQP], bf16, tag="t2")
            dots = atp.tile([NP, PQ], f32, tag="dots")
            s4n = atp.tile([NP, J], bf16, tag="s4n")
            recip = atp.tile([NB, 1], bf16, tag="recip")

            # residual stream: x^T as (128, 4*32), pure bf16
            xT_b = apool.tile([128, 4 * NB], bf16, tag="xb")
            nc.vector.memset(xT_b, 0.0)
            xT_mid = xT_b

            def xsl(t, kt):
                return t[:, kt * NB : (kt + 1) * NB]

            t1_3 = t1[:, :].rearrange("p (a b) -> p a b", b=JP)
            t2_3 = t2[:, :].rearrange("p (a b) -> p a b", b=PQP)

            prev = None  # (wb, uT, y_ps, sign) of layer l-1 pending fc1
            for l in range(L):
                wb = wpool.tile([128, 3 * 2048], fp8, tag="wb")
                nc.sync.dma_start(out=wb, in_=d_wbig[l, :, :])

                def wtile(mat, o, kt, wbx=None):
                    wbx = wb if wbx is None else wbx
                    c = ((mat * 4 + o) * 4 + kt) * 128
                    return wbx[:, c : c + 128]

                qk_l = qkfq[:, l * 8 * J : l * 8 * J + 4 * J]
                fq_l = qkfq[:, l * 8 * J + 4 * J : (l + 1) * 8 * J]
                aab = abs(alpha[l])

                if l == 0:
                    # layer 0 attention+fin folded on host (x=0): x_mid is a const
                    xT_mid = xmid0
                    tv = xmid0
                else:
                    # ---- g (32,19) then replicate to 128 group-major rows ----
                    g_ps = pt.tile([NB, J], f32, tag="sp")
                    wb_p, uT_p, y_p, sgn_p = prev
                    nc.tensor.matmul(
                        g_ps, ones1, gconst[:, l * J : (l + 1) * J],
                        start=True, stop=False,
                    )
                    for kt in range(4):
                        nc.tensor.matmul(
                            g_ps, xsl(xT_mid, kt),
                            qk_l[:, kt * J : (kt + 1) * J],
                            start=False, stop=False,
                        )
                    for kt in range(4):
                        nc.tensor.matmul(
                            g_ps, xsl(uT_p, kt),
                            fq_l[:, kt * J : (kt + 1) * J],
                            start=False, stop=(kt == 3),
                        )
                    nc.vector.tensor_copy(g_sb, g_ps)
                    g3_ps = pt.tile([NP, J], f32, tag="sp")
                    nc.tensor.matmul(g3_ps, e2tp, g_sb, start=True, stop=True)
                    nc.scalar.activation(g3[:, 0:J], g3_ps, Act.Copy)

                # ---- deferred fc1 of layer l-1 (hides under attention) ----
                if l > 0:
                    for o in range(4):
                        for kt in range(4):
                            nc.tensor.matmul(
                                xsl(y_p, o), wtile(2, o, kt, wb_p),
                                xsl(uT_p, kt),
                                start=False, stop=(kt == 3),
                            )

                # psum tiles + early bias rank-1s (no deps; run in PE slack)
                ft = pm.tile([128, 4 * NB], f32, tag="mm")
                u_ps = pm.tile([128, 4 * NB], f32, tag="mm")
                y_ps = pm.tile([128, 4 * NB], f32, tag="mm")
                for o in range(4):
                    nc.tensor.matmul(
                        xsl(u_ps, o),
                        biasall[:, l * 2 * D + o * 128 : l * 2 * D + (o + 1) * 128],
                        ones_bf, start=True, stop=False,
                    )
                    nc.tensor.matmul(
                        xsl(y_ps, o),
                        biasall[:, l * 2 * D + D + o * 128 : l * 2 * D + D + (o + 1) * 128],
                        ones_bf, start=True, stop=False,
                    )

                if l > 0:
                    # ---- dots = bfeat . g -> (128, 21) ----
                    nc.vector.tensor_tensor(
                        t1_3, bfpj4, bcast_mid(g3[:, :], PQ), op=AluOp.mult
                    )
                    nc.vector.tensor_reduce(
                        dots, t1_3, axis=mybir.AxisListType.X, op=AluOp.add
                    )
                    # fc1 residual of layer l-1 (DVE slot after dots)
                    nxb = apool.tile([128, 4 * NB], bf16, tag="xb")
                    nc.vector.tensor_tensor(
                        nxb, xT_mid, y_p,
                        op=AluOp.add if sgn_p >= 0 else AluOp.subtract,
                    )
                    xT_b = nxb
                    # fin + fc0 x-parts now that x_l is final
                    for o in range(4):
                        for kt in range(4):
                            nc.tensor.matmul(
                                xsl(ft, o), wtile(0, o, kt), xsl(xT_b, kt),
                                start=(kt == 0), stop=False,
                            )
                    for o in range(4):
                        for kt in range(4):
                            nc.tensor.matmul(
                                xsl(u_ps, o), wtile(1, o, kt), xsl(xT_b, kt),
                                start=False, stop=False,
                            )
                    with nc.allow_low_precision(reason="softmax rowsum"):
                        nc.scalar.activation(
                            e4[:, 0:PQ], dots, Act.Exp, accum_out=rs
                        )
                    rsb_ps = pt.tile([NB, 1], f32, tag="sp")
                    nc.tensor.matmul(rsb_ps, e2pf, rs, start=True, stop=True)

                    # ---- s4 = sum_p e[b,p] bfeat[b,p,j] (unnormalized) ----
                    nc.vector.tensor_tensor(
                        t2_3, bfjp4, bcast_mid(e4[:, :], J), op=AluOp.mult
                    )
                    with nc.allow_low_precision(reason="softmax recip bf16"):
                        nc.vector.reciprocal(recip, rsb_ps)
                    r3_ps = pt.tile([128, 1], f32, tag="sp")
                    nc.tensor.matmul(r3_ps, e2tp, recip, start=True, stop=True)
                    with nc.allow_low_precision(reason="attention s in bf16"):
                        nc.vector.tensor_reduce(
                            s4, t2_3, axis=mybir.AxisListType.X, op=AluOp.add
                        )
                    nc.vector.tensor_scalar_mul(s4n, s4, r3_ps[:, :])
                    # group-sum to s^T directly: (19,32) = s4n.T @ e2
                    sT_ps = pt.tile([J, NB], f32, tag="sp")
                    nc.tensor.matmul(sT_ps, s4n, e2p, start=True, stop=True)
                    nc.vector.tensor_copy(sT_buf[0:J, :], sT_ps)

                    # ---- fin tail: += s @ [sfin;cfin] (K=20, bias inside) ----
                    for o in range(4):
                        nc.tensor.matmul(
                            xsl(ft, o),
                            sfall[:, l * D + o * 128 : l * D + (o + 1) * 128],
                            sT_buf,
                            start=False, stop=True,
                        )
                    # tv = alpha*relu(ft); fc0 tv-part rides on it so the
                    # x_mid residual add leaves the critical chain
                    tv = apool.tile([128, 4 * NB], bf16, tag="tv")
                    nc.vector.tensor_scalar(
                        tv, ft, 0.0, float(alpha[l]),
                        op0=AluOp.max, op1=AluOp.mult,
                    )
                for o in range(4):
                    for kt in range(4):
                        nc.tensor.matmul(
                            xsl(u_ps, o), wtile(1, o, kt), xsl(tv, kt),
                            start=False, stop=(kt == 3),
                        )
                if l > 0:
                    nmid = apool.tile([128, 4 * NB], bf16, tag="xb")
                    nc.vector.tensor_tensor(nmid, xT_b, tv, op=AluOp.add)
                    xT_mid = nmid
                nuT = apool.tile([128, 4 * NB], bf16, tag="uT")
                nc.scalar.activation(nuT, u_ps, Act.Relu, scale=aab)
                uT = nuT
                # fc1 matmuls deferred to next layer (after its g matmuls)
                prev = (wb, uT, y_ps, 1.0 if alpha[l] >= 0 else -1.0)

            # flush last layer's fc1 + residual
            wb_p, uT_p, y_p, sgn_p = prev
            for o in range(4):
                for kt in range(4):
                    nc.tensor.matmul(
                        xsl(y_p, o), wtile(2, o, kt, wb_p), xsl(uT_p, kt),
                        start=False, stop=(kt == 3),
                    )
            nxb = apool.tile([128, 4 * NB], bf16, tag="xb")
            nc.vector.tensor_tensor(
                nxb, xT_mid, y_p,
                op=AluOp.add if sgn_p >= 0 else AluOp.subtract,
            )
            xT_b = nxb

            # ---- head: logits = log_softmax((x @ head_w) @ posT) ----
            zT_ps = pt.tile([POSD, NB], f32, tag="sp")
            for kt in range(4):
                nc.tensor.matmul(
                    zT_ps, whead[:, kt * POSD : (kt + 1) * POSD], xsl(xT_b, kt),
                    start=(kt == 0), stop=(kt == 3),
                )
            zT = apool.tile([POSD, NB], bf16, tag="zT")
            nc.vector.tensor_copy(zT, zT_ps)
            lg_ps = pt.tile([NB, P], f32, tag="sp")
            nc.tensor.matmul(lg_ps, zT, wpost, start=True, stop=True)
            lg = apool.tile([NB, P], f32, tag="lg")
            nc.scalar.activation(lg, lg_ps, Act.Copy)
            mx = apool.tile([NB, 1], f32, tag="mx")
            nc.vector.tensor_reduce(
                mx, lg[:, :], axis=mybir.AxisListType.X, op=AluOp.max
            )
            negmx = apool.tile([NB, 1], f32, tag="nmx")
            nc.vector.tensor_scalar_mul(negmx, mx, -1.0)
            ex = apool.tile([NB, P], f32, tag="ex")
            sume = apool.tile([NB, 1], f32, tag="sume")
            nc.scalar.activation(
                ex, lg, Act.Exp, bias=negmx[:, :], accum_out=sume
            )
            lse = apool.tile([NB, 1], f32, tag="lse")
            nc.scalar.activation(lse, sume, Act.Ln)
            c = apool.tile([NB, 1], f32, tag="c")
            nc.vector.tensor_add(c, mx, lse)
            outf = apool.tile([NB, P], f32, tag="outf")
            nc.vector.tensor_scalar(
                outf, lg[:, :], c[:, :], None, op0=AluOp.subtract
            )
            nc.sync.dma_start(out=d_out[:, :], in_=outf)

    nc.finalize()
    return nc


def kernel(**inputs):
    inp = {k: np.asarray(v, dtype=np.float32) for k, v in inputs.items()}
    pos = _positions()
    bfeat = _prepare(inp["obs"], pos)  # (256, 81, 19)
    qk_w, qk_b, afin, sfin, cfin = _fold(inp)
    alpha = inp["alpha"].astype(np.float32)

    wbig = np.concatenate(
        [_ktile_lhsT(afin), _ktile_lhsT(inp["fc0_w"]), _ktile_lhsT(inp["fc1_w"])],
        axis=2,
    ).astype(fp8_np)  # (L, 128, 6144)

    # g pipelining folds: fq_l = sign(a_{l-1}) * fc1_{l-1} @ qk_l
    fq = np.zeros((L, D, J), np.float32)
    gconst = qk_b.copy()
    for l in range(1, L):
        sgn = 1.0 if alpha[l - 1] >= 0 else -1.0
        fq[l] = sgn * (inp["fc1_w"][l - 1] @ qk_w[l])
        gconst[l] = qk_b[l] + alpha[l - 1] * (inp["fc1_b"][l - 1] @ qk_w[l])
    qkfq = np.zeros((128, L * 8 * J), np.float32)
    for l in range(L):
        qkfq[:, l * 8 * J : l * 8 * J + 4 * J] = (
            qk_w[l].reshape(4, 128, J).transpose(1, 0, 2).reshape(128, 4 * J)
        )
        qkfq[:, l * 8 * J + 4 * J : (l + 1) * 8 * J] = (
            fq[l].reshape(4, 128, J).transpose(1, 0, 2).reshape(128, 4 * J)
        )

    sfin_aug = np.concatenate([sfin, cfin[:, None, :]], axis=1)  # (L, 20, 512)
    sfall = np.ascontiguousarray(
        sfin_aug.transpose(1, 0, 2)
    ).reshape(JP, L * D).astype(bf16_np)

    biasall = np.zeros((1, L * 2 * D), np.float32)
    for l in range(L):
        biasall[0, l * 2 * D : l * 2 * D + D] = inp["fc0_b"][l]
        biasall[0, l * 2 * D + D : (l + 1) * 2 * D] = (
            (1.0 if alpha[l] >= 0 else -1.0) * alpha[l] * inp["fc1_b"][l]
        )
    gconst_v = gconst.reshape(1, L * J).astype(np.float32)
    whead = (
        inp["head_w"].reshape(4, 128, POSD).transpose(1, 0, 2)
        .reshape(128, 4 * POSD).astype(bf16_np)
    )

    # group-major constants: partition p = gg*32 + b, cell = gg*21 + pq
    e2 = np.zeros((NP, NB), np.float32)
    for gg in range(GG):
        for b in range(NB):
            e2[gg * NB + b, b] = 1.0

    in_maps = []
    for cc in range(NCORES):
        bf = bfeat[cc * NB : (cc + 1) * NB]          # (32, 81, 19)
        # pad cells 81..83 with zeros, cell c -> (gg=c//21, pq=c%21)
        bfp = np.zeros((NB, GG * PQ, J), np.float32)
        bfp[:, :P, :] = bf
        bfg = bfp.reshape(NB, GG, PQ, J).transpose(1, 0, 2, 3)  # (gg,b,pq,j)
        cpk = np.zeros((128, C16_END), np.float32)
        bfpj = np.zeros((GG, NB, PQ, JP), np.float32)
        bfpj[:, :, :, :J] = bfg
        # pad-cell kill switch: -40 in the j-pad lane (g3 pad col is 1.0)
        for c in range(P, GG * PQ):
            bfpj[c // PQ, :, c % PQ, J] = NEGBIG
        cpk[:, C_BFPJ:C_BFJP] = bfpj.reshape(NP, PQ * JP)
        bfjp = np.zeros((GG, NB, J, PQP), np.float32)
        bfjp[:, :, :, :PQ] = bfg.transpose(0, 1, 3, 2)
        cpk[:, C_BFJP:C_POST] = bfjp.reshape(NP, J * PQP)
        cpk[:POSD, C_POST:C_E2] = pos.reshape(P, POSD).T
        cpk[:, C_E2:C_E2T] = e2
        cpk[:NB, C_E2T:C16_END] = e2.T
        # layer-0 folded on host (x=0): x_mid0 = a0*relu(s0@sfin0 + cfin0)
        dots0 = bf @ gconst[0]                      # (32, 81)
        e0 = np.exp(dots0)
        attn0 = e0 / e0.sum(1, keepdims=True)
        s0 = np.einsum("bp,bpj->bj", attn0, bf)
        ft0 = s0 @ sfin[0] + cfin[0]
        xm0 = alpha[0] * np.maximum(ft0, 0.0)       # (32, 512)
        xmid0 = np.ascontiguousarray(
            xm0.reshape(NB, 4, 128).transpose(2, 1, 0)
        ).reshape(128, 4 * NB)
        in_maps.append({
            "cpk16": cpk.astype(bf16_np), "e2pf": e2, "xmid0": xmid0.astype(bf16_np),
            "wbig": wbig, "qkfq": qkfq.astype(bf16_np), "sfall": sfall,
            "bias": biasall.astype(bf16_np), "gconst": gconst_v, "whead": whead,
        })

    nc = _build_nc([float(a) for a in alpha])
    res = run_bass_kernel_spmd(nc, in_maps, core_ids=list(range(NCORES)))
    out = np.concatenate([r["out"] for r in res.results], axis=0)  # (256, 81)
    return out.astype(np.float32)
